# revision 5
# baseline (speedup 1.0000x reference)
"""HT2SPHERE kernel for Trainium2 (8 NeuronCores).

Computation: out[b,c,s] = sum_v input[b,c,ht_idx[v]] * weight[v] * [sph_idx[v]==s]

Strategy: vote_mapping is static geometry (a precomputed LUT in the original
model), so the host "inspector" folds the 1M votes into a dense vote matrix
M[ht, s] = sum of weights of votes mapping ht -> s.  Then
    out[bc, :] = in_flat[bc, :] @ M
On device this is a dense [256 x 16384] @ [16384 x 16384] matmul.  We shard
M's sphere columns across the 8 cores (2048 columns each); the input
(pre-transposed to lhsT layout on host) is replicated.  Each core streams its
128 MB M-slice once from HBM (the roofline term) while accumulating all of
K=16384 into 8 PSUM banks ([2 bc-blocks x 4 n-blocks] of [128 x 512]).
"""

import numpy as np

import concourse.bacc as bacc
import concourse.mybir as mybir
import concourse.tile as tile
from concourse import bass_utils

HW = 16384  # H*W Hough-grid cells (contraction dim K)
S = 16384  # sphere bins
BC = 256  # B*C rows
NCORES = 8
NS = S // NCORES  # sphere columns per core (2048)
P = 128
KT = HW // P  # 128 k-tiles
F32 = mybir.dt.float32

LAST_RESULTS = None  # stashed BassKernelResults for test harness inspection
LAST_IN_MAPS = None  # stashed per-core input maps for test-harness timing

_NC_CACHE = None


def _build_nc(minimal=False):
    nc = bacc.Bacc("TRN2", target_bir_lowering=False, debug=False, num_devices=NCORES)
    # lhsT host layout: [KT, P, BC]  (= in_flat.T reshaped, k-tile major)
    lhsT = nc.dram_tensor("lhsT", [KT, P, BC], F32, kind="ExternalInput").ap()
    m = nc.dram_tensor("m", [HW, NS], F32, kind="ExternalInput").ap()
    out = nc.dram_tensor("out", [BC, NS], F32, kind="ExternalOutput").ap()

    if minimal:
        # near-no-op baseline used to measure transfer/dispatch overhead
        with tile.TileContext(nc) as tc:
            with tc.tile_pool(name="sb", bufs=1) as pool:
                t = pool.tile([P, 512], F32)
                nc.sync.dma_start(t[:], m[:P, :512])
                nc.sync.dma_start(out[:P, :512], t[:])
        nc.compile()
        return nc

    with tile.TileContext(nc) as tc:
        with (
            tc.tile_pool(name="lhs", bufs=1) as lhs_pool,
            tc.tile_pool(name="mt", bufs=4) as m_pool,
            tc.tile_pool(name="ot", bufs=1) as o_pool,
            tc.tile_pool(name="ps", bufs=1, space="PSUM") as ps_pool,
        ):
            lhs_sb = lhs_pool.tile([P, KT, BC], F32)  # 16.8 MB resident
            # load lhsT in 8 chunks so early matmuls can start sooner
            CH = KT // 8
            for kc in range(8):
                nc.sync.dma_start(
                    lhs_sb[:, kc * CH : (kc + 1) * CH, :],
                    lhsT[kc * CH : (kc + 1) * CH, :, :].rearrange("ko p m -> p ko m"),
                )

            psums = [
                [ps_pool.tile([P, 512], F32, name=f"psum_{mi}_{ni}") for ni in range(4)]
                for mi in range(2)
            ]
            for ki in range(KT):
                mtile = m_pool.tile([P, NS], F32)  # 1 MB
                nc.sync.dma_start(mtile[:], m[ki * P : (ki + 1) * P, :])
                for mi in range(2):
                    for ni in range(4):
                        nc.tensor.matmul(
                            psums[mi][ni][:],
                            lhs_sb[:, ki, mi * P : (mi + 1) * P],
                            mtile[:, ni * 512 : (ni + 1) * 512],
                            start=(ki == 0),
                            stop=(ki == KT - 1),
                        )
            out_sb = o_pool.tile([P, 2, NS], F32)
            for mi in range(2):
                for ni in range(4):
                    nc.vector.tensor_copy(
                        out=out_sb[:, mi, ni * 512 : (ni + 1) * 512],
                        in_=psums[mi][ni][:],
                    )
            nc.sync.dma_start(out.rearrange("(mo p) n -> p mo n", p=P), out_sb[:])

    nc.compile()
    return nc


def kernel(input, vote_mapping, sphere_size):
    global LAST_RESULTS, LAST_IN_MAPS, _NC_CACHE
    input = np.asarray(input, dtype=np.float32)
    vote_mapping = np.asarray(vote_mapping, dtype=np.float32)
    assert int(sphere_size) == S
    b, c, h, w = input.shape
    assert b * c == BC and h * w == HW

    # --- host inspector: fold votes into dense vote matrix M [HW, S] ---
    ht = vote_mapping[:, 0].astype(np.int32)
    wt = vote_mapping[:, 1].astype(np.float32)
    sph = vote_mapping[:, 2].astype(np.int32)
    M = np.zeros((HW, S), dtype=np.float32)
    np.add.at(M, (ht, sph), wt)

    in_flat = input.reshape(BC, HW)
    lhsT = np.ascontiguousarray(in_flat.T).reshape(KT, P, BC)

    if _NC_CACHE is None:
        _NC_CACHE = _build_nc()
    nc = _NC_CACHE

    in_maps = [
        {"lhsT": lhsT, "m": np.ascontiguousarray(M[:, k * NS : (k + 1) * NS])}
        for k in range(NCORES)
    ]
    del M
    LAST_IN_MAPS = in_maps
    res = bass_utils.run_bass_kernel_spmd(nc, in_maps, core_ids=list(range(NCORES)))
    LAST_RESULTS = res
    out = np.concatenate([res.results[k]["out"] for k in range(NCORES)], axis=1)
    return out.reshape(b, c, S)
